# revision 9
# baseline (speedup 1.0000x reference)
"""BERT input representation kernel for 8 TRN2 NeuronCores.

Math (reference):
    x1  = x @ W_emb + b_emb                      # [B,S,D]
    seg = einsum('bnsd,s->bnd', x1.reshape(B,S/8,8,D), w_seg) + b_seg
    out = (x1.reshape(...) + seg[:,:,None,:]).reshape(B,S,D) + PE(S,D)

Folded form used here (exact algebra):
    out[b,s,:] = (A @ x[b])[s,:] @ W_emb + bias[s,:]
where A = I + blockdiag(ones(8,1) @ w_seg[None,:]) mixes rows within each
8-row segment, and bias[s,:] = PE[s,:] + b_emb*(1 + sum(w_seg)) + b_seg.

Sharding: pure data-parallel over batch; each of 8 cores handles 8 batches
(4096 rows = 32 row-tiles of 128 rows). Device schedule per core:
  - load all of x [128, 32*64] f32 (host did a layout-only rearrange so
    partition p holds row p of every tile), cast to bf16 on DVE
  - phase 1 (also serves as PE warm-up burst): 16 matmuls, each computing
    x~^T for a PAIR of row-tiles:  psum[128, 128] = x2.T @ A^T where x2
    stacks two tiles' 64 features; ACT copies psum -> resident xt bf16
  - phase 2, per pair of row-tiles (16 groups):
      PE: preload bias (high 512 cols) into PSUM via identity matmul,
          then out_psum = xt.T @ W (low half start=True, high half
          accumulates onto the preloaded bias)
      DVE: out_sbuf[low 512]  = out_psum[low]  + bias   (tensor_add)
      ACT: out_sbuf[high 512] = out_psum[high]          (plain copy)
      one 1 MiB store per group (two row-tiles) on the sync HWDGE ring
"""

import sys

if "/opt/trn_rl_repo" not in sys.path:
    sys.path.insert(0, "/opt/trn_rl_repo")

import ml_dtypes
import numpy as np

import concourse.bacc as bacc
import concourse.mybir as mybir
import concourse.tile as tile
from concourse.bass_utils import run_bass_kernel_spmd

B, S, F, D, SEG = 64, 512, 64, 1024, 8
N_CORES = 8
B_LOC = B // N_CORES          # batches per core
ROWS = B_LOC * S              # 4096 rows per core
TILE_P = 128                  # rows per tile
N_TILES = ROWS // TILE_P      # 32
N_PAIR = N_TILES // 2         # 16 tile-pairs
N_BIAS = S // TILE_P          # 4 distinct bias row-tiles
HD = D // 2                   # 512

_NC_CACHE = None


def _build_nc():
    nc = bacc.Bacc("TRN2", target_bir_lowering=False, debug=False,
                   num_devices=N_CORES)
    # x pre-rearranged on host (layout only): xr[p, i*F:(i+1)*F] = x[i*128+p]
    x_d = nc.declare_dram_parameter("x", [TILE_P, N_TILES * F],
                                    mybir.dt.float32, isOutput=False)
    at_d = nc.declare_dram_parameter("at", [TILE_P, TILE_P],
                                     mybir.dt.bfloat16, isOutput=False)
    id_d = nc.declare_dram_parameter("ident", [TILE_P, TILE_P],
                                     mybir.dt.bfloat16, isOutput=False)
    # W stacked twice on host: partitions 0-63 and 64-127 both hold W,
    # so mains with lhsT at base_partition 64 have a matching-base rhs.
    w_d = nc.declare_dram_parameter("w", [2 * F, D], mybir.dt.bfloat16,
                                    isOutput=False)
    # bias rearranged: [128, 4*D], column block j = bias rows j*128..j*128+127
    b_d = nc.declare_dram_parameter("bias", [TILE_P, N_BIAS * D],
                                    mybir.dt.bfloat16, isOutput=False)
    out_d = nc.declare_dram_parameter("out", [ROWS, D], mybir.dt.float32,
                                      isOutput=True)

    with tile.TileContext(nc) as tc:
        with (
            tc.tile_pool(name="const", bufs=1) as cpool,
            tc.tile_pool(name="xin", bufs=2) as xpool,
            tc.tile_pool(name="xbf", bufs=2) as xbpool,
            tc.tile_pool(name="outp", bufs=3) as opool,
            tc.tile_pool(name="psum", bufs=3, space="PSUM") as pso,
        ):
            # bias split per row-block so early groups are gated only on
            # the blocks they use
            bias_sb = cpool.tile([TILE_P, N_BIAS * D], mybir.dt.bfloat16)
            for jb in range(N_BIAS):
                nc.scalar.dma_start(bias_sb[:, jb * D:(jb + 1) * D],
                                    b_d[:, jb * D:(jb + 1) * D])
            w_sb = cpool.tile([2 * F, D], mybir.dt.bfloat16)
            nc.scalar.dma_start(w_sb[:], w_d[:])
            at_sb = cpool.tile([TILE_P, TILE_P], mybir.dt.bfloat16)
            nc.scalar.dma_start(at_sb[:], at_d[:])
            i_sb = cpool.tile([TILE_P, TILE_P], mybir.dt.bfloat16)
            nc.scalar.dma_start(i_sb[:], id_d[:])

            # resident x~^T (bf16): xt_sb[64u+f, 128*pr+n] = x~[2pr+u, n, f]
            xt_sb = cpool.tile([TILE_P, N_PAIR * TILE_P], mybir.dt.bfloat16)

            # waves: load an x chunk, build x~^T for the wave's pairs
            # (phase 1), then matmul+bias+store those groups (phase 2).
            # Small first waves so the first output store launches early.
            WAVES = [1, 1, 2, 4, 4, 4]
            pr0 = 0
            for wn, wp in enumerate(WAVES):
                c0, cw = pr0 * TILE_P, wp * TILE_P   # x cols of this wave
                xc = xpool.tile([TILE_P, 512], mybir.dt.float32, name="xc",
                                tag="xc")
                nc.sync.dma_start(xc[:, 0:cw], x_d[:, c0:c0 + cw])
                xcb = xbpool.tile([TILE_P, 512], mybir.dt.bfloat16,
                                  name="xcb", tag="xcb")
                nc.vector.tensor_copy(xcb[:, 0:cw], xc[:, 0:cw])

                ps_x = pso.tile([TILE_P, 512], mybir.dt.float32,
                                name="ps_x", tag="ps_x", bufs=2)
                for k in range(wp):
                    nc.tensor.matmul(ps_x[:, 128 * k:128 * (k + 1)],
                                     xcb[:, 128 * k:128 * (k + 1)],
                                     at_sb[:], start=True, stop=True)
                nc.scalar.copy(xt_sb[:, c0:c0 + cw], ps_x[:, 0:cw])

                for j in range(pr0, pr0 + wp):
                    o_sb = opool.tile([TILE_P, 2 * D], mybir.dt.float32,
                                      name="o_sb")
                    ps0 = pso.tile([TILE_P, D], mybir.dt.float32,
                                   name="ps0", tag="ps", bufs=3)
                    ps1 = pso.tile([TILE_P, D], mybir.dt.float32,
                                   name="ps1", tag="ps", bufs=3)
                    pss = (ps0, ps1)
                    jbs = ((2 * j) % N_BIAS, (2 * j + 1) % N_BIAS)
                    lhss = tuple(
                        xt_sb[64 * u:64 * (u + 1), 128 * j:128 * (j + 1)]
                        for u in range(2))
                    # bias preload into the high halves (shared I weights)
                    for u in range(2):
                        nc.tensor.matmul(
                            pss[u][:, HD:D], i_sb[:],
                            bias_sb[:, jbs[u] * D + HD:(jbs[u] + 1) * D],
                            start=True, stop=False)
                    # mains: u=0 on PE rows 0-63, u=1 on rows 64-127 —
                    # disjoint row groups execute concurrently
                    for u in range(2):
                        nc.tensor.matmul(pss[u][:, HD:D], lhss[u],
                                         w_sb[64 * u:64 * u + F, HD:D],
                                         start=False, stop=True)
                    for u in range(2):
                        nc.tensor.matmul(pss[u][:, 0:HD], lhss[u],
                                         w_sb[64 * u:64 * u + F, 0:HD],
                                         start=True, stop=True)
                    for u in range(2):
                        nc.vector.tensor_add(
                            o_sb[:, u * D:u * D + HD], pss[u][:, 0:HD],
                            bias_sb[:, jbs[u] * D:jbs[u] * D + HD])
                        nc.scalar.copy(o_sb[:, u * D + HD:(u + 1) * D],
                                       pss[u][:, HD:D])
                    dram = out_d[j * 256:(j + 1) * 256, :].rearrange(
                        "(a p) d -> p a d", a=2, p=TILE_P)
                    nc.sync.dma_start(dram, o_sb[:].rearrange(
                        "p (a d) -> p a d", a=2))
                pr0 += wp
    nc.compile()
    return nc


def _host_constants(W_emb, b_emb, w_seg, b_seg):
    # sinusoidal positional encoding, float32, same formula as the reference
    pos = np.arange(S, dtype=np.float32)[:, None]
    div = np.exp(np.arange(0, D, 2, dtype=np.float32)
                 * (-np.log(10000.0) / D)).astype(np.float32)
    ang = pos * div
    pe = np.zeros((S, D), np.float32)
    pe[:, 0::2] = np.sin(ang)
    pe[:, 1::2] = np.cos(ang)

    bias = (pe + b_emb[None, :] * (np.float32(1.0) + w_seg.sum())
            + b_seg[0]).astype(np.float32)
    # rearrange to [128, 4*D]: column block j holds bias rows j*128..j*128+127
    bias_r = np.ascontiguousarray(
        bias.reshape(N_BIAS, TILE_P, D).transpose(1, 0, 2).reshape(
            TILE_P, N_BIAS * D)).astype(ml_dtypes.bfloat16)

    blk = np.eye(SEG, dtype=np.float32) + w_seg[:, None] * np.ones(
        (1, SEG), np.float32)
    at = np.kron(np.eye(TILE_P // SEG, dtype=np.float32), blk).astype(
        ml_dtypes.bfloat16)

    ident = np.eye(TILE_P, dtype=np.float32).astype(ml_dtypes.bfloat16)
    wb = np.ascontiguousarray(
        np.vstack([W_emb, W_emb])).astype(ml_dtypes.bfloat16)
    return at, ident, wb, bias_r


def _prepare_in_maps(x, W_emb, b_emb, w_seg, b_seg):
    x = np.ascontiguousarray(np.asarray(x, dtype=np.float32))
    W_emb = np.asarray(W_emb, dtype=np.float32)
    b_emb = np.asarray(b_emb, dtype=np.float32)
    w_seg = np.asarray(w_seg, dtype=np.float32)
    b_seg = np.asarray(b_seg, dtype=np.float32)

    at, ident, wb, bias_r = _host_constants(W_emb, b_emb, w_seg, b_seg)

    in_maps = []
    for c in range(N_CORES):
        xs = x[c * B_LOC:(c + 1) * B_LOC].reshape(ROWS, F)
        # layout-only rearrange: [32 tiles, 128 rows, F] -> [128, 32*F]
        xr = np.ascontiguousarray(
            xs.reshape(N_TILES, TILE_P, F).transpose(1, 0, 2).reshape(
                TILE_P, N_TILES * F))
        in_maps.append({"x": xr, "at": at, "ident": ident, "w": wb,
                        "bias": bias_r})
    return in_maps


def kernel(x, W_emb, b_emb, w_seg, b_seg):
    in_maps = _prepare_in_maps(x, W_emb, b_emb, w_seg, b_seg)

    global _NC_CACHE
    if _NC_CACHE is None:
        _NC_CACHE = _build_nc()

    res = run_bass_kernel_spmd(_NC_CACHE, in_maps,
                               core_ids=list(range(N_CORES)))
    out = np.concatenate(
        [np.asarray(res.results[c]["out"]).reshape(B_LOC, S, D)
         for c in range(N_CORES)], axis=0)
    return out
